# revision 27
# baseline (speedup 1.0000x reference)
"""Multi-head attention + residual + LayerNorm on 8 Trainium2 NeuronCores.

Reference computation (B=2, S=2048, D=1024, H=16, HD=64):
    q,k,v = split_heads(x@Wq+bq), ...       # [B,H,S,HD]
    attn  = softmax(q k^T / sqrt(HD))
    out   = (attn v) merged -> [B,S,D] @ Wp + bp
    y     = LayerNorm(x + out) * gamma + beta

Sharding: 8 cores = 2 batches x 4 query-slices of 512 rows.
Each core computes QKV projections for its 512-row slice; K^T (bf16) and
V (fp8e4) slices are AllGathered across the 4 cores of the same batch in
8 per-head-pair pieces fired as soon as each head pair's K/V is ready, so
attention pipelines behind the gathers. Each core runs attention for all
16 heads restricted to its 512 queries, then projection + residual + LN.

Softmax numerators are fp8e4: even-ish key chunks go through the scalar
engine's exact ACT exp, the rest through a one-pass Schraudolph bit-trick
on the vector engine (f32 -> uint8 code with round+saturate, bitcast as
fp8e4). attn@V runs as fp8 DoubleRow matmuls (two 128-key chunks per
pass). Residual + LayerNorm in f32.
"""

import os

import ml_dtypes
import numpy as np

import concourse.bacc as bacc
import concourse.tile as tile
from concourse import mybir
from concourse.bass_utils import run_bass_kernel_spmd

B, S, D, H, HD = 2, 2048, 1024, 16, 64
EPS = 1e-5
NCORES = 8
SL = S // 4          # 512 query rows per core
GROUPS = [[0, 1, 2, 3], [4, 5, 6, 7]]
BF = mybir.dt.bfloat16
F32 = mybir.dt.float32
FP8 = mybir.dt.float8e4
U8 = mybir.dt.uint8
I16 = mybir.dt.int16
Act = mybir.ActivationFunctionType
Alu = mybir.AluOpType
DR = mybir.MatmulPerfMode.DoubleRow

# exp(s*0.125 + BSH); uniform shift cancels in softmax, keeps fp8 in range
BSH = -1.2
LOG2E = 1.4426950408889634
# Schraudolph to bf16 codes: i16 = round(s*EC1 + EC2), bitcast bf16
EC1 = 128 * LOG2E * 0.125
EC2 = 16256.0 + 128 * LOG2E * BSH + 5.5
# kc chunks handled by the scalar engine's exact ACT exp (rest: DVE trick)
SCALAR_KCS = {0, 2, 4, 6, 8, 10, 12, 14}

KB = 128 * SL            # 65536 bf16 elems: K^T block per head pair
VB = 128 * SL // 2       # V block as bf16-slot count (fp8 bytes = 128*SL)
KVB = KB + VB            # 98304


def build_program():
    nc = bacc.Bacc("TRN2", target_bir_lowering=False, debug=False,
                   num_devices=NCORES)

    # ---- I/O ----
    xT_d = nc.dram_tensor("xT", [D, SL], BF, kind="ExternalInput")
    xq_d = nc.dram_tensor("xq", [SL, D], F32, kind="ExternalInput")
    wq_d = nc.dram_tensor("wq", [D, D], BF, kind="ExternalInput")
    wk_d = nc.dram_tensor("wk", [D, D], BF, kind="ExternalInput")
    wv_d = nc.dram_tensor("wv", [D, D], BF, kind="ExternalInput")
    wp_d = nc.dram_tensor("wp", [D, D], FP8, kind="ExternalInput")
    bq_d = nc.dram_tensor("bq", [D], F32, kind="ExternalInput")
    bk_d = nc.dram_tensor("bk", [D], F32, kind="ExternalInput")
    bv_d = nc.dram_tensor("bv", [D], F32, kind="ExternalInput")
    bp_d = nc.dram_tensor("bp", [D], BF, kind="ExternalInput")
    gamma_d = nc.dram_tensor("gamma", [D], F32, kind="ExternalInput")
    beta_d = nc.dram_tensor("beta", [D], F32, kind="ExternalInput")
    y_d = nc.dram_tensor("y", [SL, D], F32, kind="ExternalOutput")

    import concourse.bass as bass

    def bcast_ap(dram_t, parts=128):
        # replicate a [D] dram vector across `parts` partitions
        return bass.AP(tensor=dram_t, offset=0, ap=[[0, parts], [1, D]])

    with tile.TileContext(nc) as tc:
        with (
            tc.tile_pool(name="persist", bufs=1) as persist,
            tc.tile_pool(name="dram", bufs=1, space="DRAM") as dram,
        ):
            # persistent tiles
            qt_sb = persist.tile([128, 8, SL], BF)        # Q^T
            xq_sb = persist.tile([128, 4, D], F32)        # natural x slice
            wp_sb = persist.tile([128, 8, 2, 512], FP8)
            outT_sb = persist.tile([128, 8, SL], FP8)     # normalized attn out^T
            bv_bc = persist.tile([128, D], F32)
            gamma_bc = persist.tile([128, D], F32)
            beta_bc = persist.tile([128, D], F32)
            bq_sb = persist.tile([128, 8], F32)
            bk_sb = persist.tile([128, 8], F32)
            bp_sb = persist.tile([1, D], BF)
            ones_sb = persist.tile([1, 128], BF)
            eps_sb = persist.tile([128, 1], F32)
            bsh_sb = persist.tile([128, 1], F32)
            # V tiles, fp8, interleaved kc pairs for DoubleRow:
            # [part=key-in-chunk, kc-pair, kc-parity, 144] with
            # cols 0:64 = V_A, 64 = ones, 65:129 = V_B, 129 = ones
            vh_tiles = [persist.tile([128, 8, 2, 144], FP8, name=f"vh{i}")
                        for i in range(2)]

            # DRAM scratch per head pair: K^T bf16 (p-major, KB elems) then
            # V fp8 (s-major, VB bf16 slots) in one bf16 tensor so a single
            # AllGather moves both.
            kvb_hp = [dram.tile([1, KVB], BF, name=f"kvb{i}") for i in range(8)]
            kvg_hp = [dram.tile([4, KVB], BF, name=f"kvg{i}") for i in range(8)]
            dumb_in = dram.tile([1, 128], BF, name="dumb_in")
            dumb_out = dram.tile([4, 128], BF, name="dumb_out")
            nc.gpsimd.collective_compute(
                "AllGather", Alu.bypass, replica_groups=GROUPS,
                ins=[dumb_in[:].opt()], outs=[dumb_out[:].opt()])

            # small/early loads first (biases feed phase-1 epilogues)
            nc.sync.dma_start(bk_sb[:], bk_d.ap().rearrange("(co p) -> p co", p=128))
            nc.sync.dma_start(bv_bc[:], bcast_ap(bv_d))
            nc.sync.dma_start(bq_sb[:], bq_d.ap().rearrange("(co p) -> p co", p=128))
            nc.vector.memset(ones_sb[:], 1.0)
            nc.vector.memset(eps_sb[:], EPS)
            nc.vector.memset(bsh_sb[:], BSH)
            for t in vh_tiles:
                nc.vector.memset(t[:, :, :, 64:65], 1.0)
                nc.vector.memset(t[:, :, :, 129:130], 1.0)
            def kvb_kview(hp):
                return kvb_hp[hp][0, 0:KB].rearrange("(p s) -> p s", p=128)

            def kvb_vview(hp):
                return kvb_hp[hp][0, KB:KVB].bitcast(FP8).rearrange(
                    "(s c) -> s c", c=128)

            # ---------------- phase 1: QKV projections for this slice ----------------
            with (
                tc.tile_pool(name="ph1w", bufs=1) as ph1w,
                tc.tile_pool(name="ph1", bufs=3) as ph1,
                tc.tile_pool(name="psum1", bufs=2, space="PSUM") as psum1,
            ):
                # x^T chunks first (every matmul needs them), then weights in
                # per-head-pair-group need-order so gathers can fire early.
                xt_c = []
                for ci in range(8):
                    xt = ph1w.tile([128, SL], BF, tag=f"xt{ci}")
                    nc.sync.dma_start(xt[:], xT_d[ci * 128:(ci + 1) * 128, :])
                    xt_c.append(xt)
                wkp, wvp = {}, {}
                for g in range(4):
                    for ci in range(8):
                        wk = ph1w.tile([128, 2, 128], BF, tag=f"wk{g}_{ci}")
                        nc.sync.dma_start(
                            wk[:], wk_d[ci * 128:(ci + 1) * 128, 256 * g:256 * (g + 1)]
                            .rearrange("p (co q) -> p co q", q=128))
                        wkp[(g, ci)] = wk
                    for ci in range(8):
                        wv = ph1w.tile([128, 256], BF, tag=f"wv{g}_{ci}")
                        nc.sync.dma_start(
                            wv[:], wv_d[ci * 128:(ci + 1) * 128, 256 * g:256 * (g + 1)])
                        wvp[(g, ci)] = wv
                wq_c = []
                for ci in range(8):
                    wq = ph1w.tile([128, 8, 128], BF, tag=f"wq{ci}")
                    nc.sync.dma_start(wq[:], wq_d[ci * 128:(ci + 1) * 128, :]
                                      .rearrange("p (co q) -> p co q", q=128))
                    wq_c.append(wq)
                # phase-3 loads last on the sync ring so they don't starve
                # the phase-1 critical loads above
                nc.sync.dma_start(xq_sb[:], xq_d.ap().rearrange("(qc p) d -> p qc d", p=128))
                nc.sync.dma_start(wp_sb[:], wp_d.ap().rearrange("(hp p) (dh q) -> p hp dh q", p=128, q=512))
                nc.sync.dma_start(gamma_bc[:], bcast_ap(gamma_d))
                nc.sync.dma_start(beta_bc[:], bcast_ap(beta_d))
                nc.sync.dma_start(bp_sb[:], bp_d.ap().rearrange("(o d) -> o d", o=1))

                def k_chunk(g, half):
                    co = 2 * g + half
                    ps = psum1.tile([128, SL], F32, tag="psk", name=f"psk{co}")
                    for ci in range(8):
                        nc.tensor.matmul(ps[:], wkp[(g, ci)][:, half, :], xt_c[ci][:],
                                         start=(ci == 0), stop=(ci == 7))
                    kt_t = ph1.tile([128, SL], BF, tag="kt", name=f"ktt{co}")
                    nc.vector.tensor_scalar_add(kt_t[:], ps[:], bk_sb[:, co:co + 1])
                    nc.gpsimd.dma_start(kvb_kview(co), kt_t[:])

                def v_pair(g):
                    ps = psum1.tile([128, 4, 256], F32, tag="psv", name=f"psv{g}")
                    for sc in range(4):
                        for ci in range(8):
                            nc.tensor.matmul(ps[:, sc, :],
                                             xt_c[ci][:, sc * 128:(sc + 1) * 128],
                                             wvp[(g, ci)][:],
                                             start=(ci == 0), stop=(ci == 7))
                    v_t = ph1.tile([128, 4, 256], FP8, tag="vt", name=f"vtt{g}")
                    with nc.allow_low_precision("V in fp8 for DoubleRow PV"):
                        for sc in range(4):
                            nc.vector.tensor_add(v_t[:, sc, :], ps[:, sc, :],
                                                 bv_bc[:, 256 * g:256 * (g + 1)])
                    for half in range(2):
                        hp = 2 * g + half
                        vv = kvb_vview(hp)
                        for sc in range(4):
                            nc.gpsimd.dma_start(
                                vv[sc * 128:(sc + 1) * 128, :],
                                v_t[:, sc, 128 * half:128 * (half + 1)])

                def fire_cc(hp):
                    nc.gpsimd.collective_compute(
                        "AllGather", Alu.bypass, replica_groups=GROUPS,
                        ins=[kvb_hp[hp][:].opt()], outs=[kvg_hp[hp][:].opt()])

                for g in range(4):
                    k_chunk(g, 0)
                    k_chunk(g, 1)
                    v_pair(g)
                    fire_cc(2 * g)
                    fire_cc(2 * g + 1)

                # Q^T (local only; overlaps the gathers)
                for co in range(8):
                    ps = psum1.tile([128, SL], F32, tag="psk", name=f"psq{co}")
                    for ci in range(8):
                        nc.tensor.matmul(ps[:], wq_c[ci][:, co, :], xt_c[ci][:],
                                         start=(ci == 0), stop=(ci == 7))
                    nc.vector.tensor_scalar_add(qt_sb[:, co, :], ps[:], bq_sb[:, co:co + 1])

            # ---------------- phase 2: attention, one head pair at a time ----------------
            with (
                tc.tile_pool(name="kv", bufs=2) as kv,
                tc.tile_pool(name="expp", bufs=2) as expp,
                tc.tile_pool(name="small", bufs=3) as small,
                tc.tile_pool(name="ps_sc", bufs=2, space="PSUM") as ps_sc,
                tc.tile_pool(name="ps_o", bufs=1, space="PSUM") as ps_o,
                tc.tile_pool(name="ps_rb", bufs=1, space="PSUM") as ps_rb,
            ):
                for hp in range(8):
                    kth_t = kv.tile([128, 16, 128], BF, tag="kth")
                    vh_t = vh_tiles[hp % 2]
                    for j in range(4):
                        ksrc = kvg_hp[hp][j, 0:KB].rearrange(
                            "(p k4 m) -> p k4 m", p=128, m=128)
                        nc.sync.dma_start(kth_t[:, 4 * j:4 * (j + 1), :], ksrc)
                        vsrc = kvg_hp[hp][j, KB:KVB].bitcast(FP8).rearrange(
                            "(jp o p c) -> p jp o c", jp=2, o=2, p=128)
                        nc.sync.dma_start(vh_t[:, 2 * j:2 * j + 2, :, 0:64],
                                          vsrc[:, :, :, 0:64])
                        nc.sync.dma_start(vh_t[:, 2 * j:2 * j + 2, :, 65:129],
                                          vsrc[:, :, :, 64:128])

                    exp_t = expp.tile([128, 16, 2, 512], FP8, tag="exp")
                    oAB = ps_o.tile([65, 2, SL], F32, tag="oAB")

                    def pv_pair(j):
                        for h in range(2):
                            nc.tensor.matmul(oAB[:, h, :],
                                             vh_t[:, j, :, 65 * h:65 * h + 65],
                                             exp_t[:, 2 * j:2 * j + 2, h, :],
                                             start=(j == 0), stop=(j == 7),
                                             perf_mode=DR)

                    for kc in range(16):
                        ps = ps_sc.tile([128, 2, 512], F32, tag="sc")
                        # head A on PE rows 0-63, head B on rows 64-127 (row-tiled)
                        nc.tensor.matmul(ps[:, 0, :], kth_t[0:64, kc, :],
                                         qt_sb[0:64, hp, :], start=True, stop=True)
                        nc.tensor.matmul(ps[:, 1, :], kth_t[64:128, kc, :],
                                         qt_sb[64:128, hp, :], start=True, stop=True)
                        if kc in SCALAR_KCS:
                            nc.scalar.activation(exp_t[:, kc, :, :], ps[:], Act.Exp,
                                                 scale=0.125, bias=bsh_sb[:])
                        else:
                            # Schraudolph exp to bf16 on DVE, then gpsimd
                            # narrows to fp8 (DVE is the scarcer engine)
                            ebf = small.tile([128, 2, 512], I16, tag="ebf")
                            with nc.allow_low_precision("schraudolph fp8 exp"):
                                nc.vector.tensor_scalar(
                                    ebf[:], ps[:], EC1, EC2, Alu.mult, Alu.add)
                                nc.gpsimd.tensor_copy(exp_t[:, kc, :, :],
                                                      ebf[:].bitcast(BF))
                        # attn@V for pair j once both its kc's exp are 2 behind
                        if kc >= 3 and kc % 2 == 1:
                            pv_pair((kc - 3) // 2)
                    pv_pair(6)
                    pv_pair(7)

                    # softmax normalization: rows 0-63 = head dims, row 64 = sum(exp)
                    sAB = small.tile([1, 2, SL], F32, tag="sAB")
                    nc.scalar.copy(sAB[:], oAB[64:65, :, :])
                    rABf = small.tile([1, 2, SL], F32, tag="rABf")
                    nc.vector.reciprocal_approx_fast(out=rABf[:], in_=sAB[:])
                    rAB = small.tile([1, 2, SL], BF, tag="rAB")
                    with nc.allow_low_precision("softmax scale in bf16"):
                        nc.vector.tensor_copy(rAB[:], rABf[:])
                    rbA = ps_rb.tile([64, SL], F32, tag="rbA")
                    rbB = ps_rb.tile([64, SL], F32, tag="rbB")
                    nc.tensor.matmul(rbA[:], ones_sb[0:1, 0:64], rAB[0:1, 0, :], start=True, stop=True)
                    nc.tensor.matmul(rbB[:], ones_sb[0:1, 0:64], rAB[0:1, 1, :], start=True, stop=True)
                    # rb psum -> sbuf so the final mult reads one PSUM operand
                    rbS = small.tile([64, 2, SL], F32, tag="rbS")
                    nc.vector.tensor_copy(rbS[:, 0, :], rbA[:])
                    nc.vector.tensor_copy(rbS[:, 1, :], rbB[:])
                    with nc.allow_low_precision("attn out in fp8 for DR proj"):
                        nc.vector.scalar_tensor_tensor(
                            out=outT_sb[0:64, hp, :], in0=oAB[0:64, 0, :], scalar=1.0,
                            in1=rbS[:, 0, :], op0=Alu.bypass, op1=Alu.mult)
                        tmpB = small.tile([64, SL], FP8, tag="tmpB")
                        nc.vector.scalar_tensor_tensor(
                            out=tmpB[:], in0=oAB[0:64, 1, :], scalar=1.0,
                            in1=rbS[:, 1, :], op0=Alu.bypass, op1=Alu.mult)
                    nc.gpsimd.dma_start(outT_sb[64:128, hp, :], tmpB[:])

            # ---------------- phase 3: out-projection + residual + LayerNorm ----------------
            with (
                tc.tile_pool(name="ph3", bufs=3) as ph3,
                tc.tile_pool(name="ph3s", bufs=4) as ph3s,
                tc.tile_pool(name="psum3", bufs=4, space="PSUM") as psum3,
            ):
                for qc in range(4):
                    y_t = ph3.tile([128, D], F32, tag="y")
                    for dh in range(2):
                        ps = psum3.tile([128, 512], F32, tag="py")
                        for t in range(4):
                            nc.tensor.matmul(ps[:],
                                             outT_sb[:, 2 * t:2 * t + 2, qc * 128:(qc + 1) * 128],
                                             wp_sb[:, 2 * t:2 * t + 2, dh, :],
                                             start=(t == 0), stop=False, perf_mode=DR)
                        # + bp via a rank-1 matmul with a ones row
                        nc.tensor.matmul(ps[:], ones_sb[0:1, :],
                                         bp_sb[0:1, dh * 512:(dh + 1) * 512],
                                         start=False, stop=True)
                        nc.vector.tensor_add(y_t[:, dh * 512:(dh + 1) * 512], ps[:],
                                             xq_sb[:, qc, dh * 512:(dh + 1) * 512])
                    # LayerNorm over D=1024
                    stats = ph3s.tile([128, 2, 6], F32, tag="stats")
                    nc.vector.bn_stats(stats[:, 0, :], y_t[:, 0:512])
                    nc.vector.bn_stats(stats[:, 1, :], y_t[:, 512:1024])
                    mv = ph3s.tile([128, 2], F32, tag="mv")
                    nc.vector.bn_aggr(mv[:], stats[:])
                    rstd = ph3s.tile([128, 1], F32, tag="rstd")
                    nc.scalar.activation(rstd[:], mv[:, 1:2], Act.Sqrt, bias=eps_sb[:])
                    nc.vector.reciprocal(rstd[:], rstd[:])
                    # y = ((y - mu) * gamma) * rstd + beta
                    nc.vector.scalar_tensor_tensor(
                        out=y_t[:], in0=y_t[:], scalar=mv[:, 0:1], in1=gamma_bc[:],
                        op0=Alu.subtract, op1=Alu.mult)
                    nc.vector.scalar_tensor_tensor(
                        out=y_t[:], in0=y_t[:], scalar=rstd[:], in1=beta_bc[:],
                        op0=Alu.mult, op1=Alu.add)
                    nc.sync.dma_start(y_d[qc * 128:(qc + 1) * 128, :], y_t[:])

    nc.compile()
    return nc


_PROGRAM = None


def _get_program():
    global _PROGRAM
    if _PROGRAM is None:
        _PROGRAM = build_program()
    return _PROGRAM


def kernel(**inputs):
    x = np.asarray(inputs["x"], np.float32)
    bf = ml_dtypes.bfloat16
    shared = {
        "wq": np.asarray(inputs["Wq"], np.float32).astype(bf),
        "wk": np.asarray(inputs["Wk"], np.float32).astype(bf),
        "wv": np.asarray(inputs["Wv"], np.float32).astype(bf),
        "wp": np.asarray(inputs["Wp"], np.float32).astype(ml_dtypes.float8_e4m3fn),
        "bq": np.asarray(inputs["bq"], np.float32),
        "bk": np.asarray(inputs["bk"], np.float32),
        "bv": np.asarray(inputs["bv"], np.float32),
        "bp": np.asarray(inputs["bp"], np.float32).astype(bf),
        "gamma": np.asarray(inputs["gamma"], np.float32),
        "beta": np.asarray(inputs["beta"], np.float32),
    }
    in_maps = []
    for c in range(NCORES):
        b, i = c // 4, c % 4
        xs = np.ascontiguousarray(x[b, i * SL:(i + 1) * SL, :])
        m = dict(shared)
        m["xT"] = np.ascontiguousarray(xs.T).astype(bf)
        m["xq"] = xs
        in_maps.append(m)

    nc = _get_program()
    trace_dir = os.environ.get("BASS_KERNEL_TRACE_DIR")
    kwargs = {}
    if trace_dir:
        kwargs = {"trace": True, "tmpdir": trace_dir}
    res = run_bass_kernel_spmd(nc, in_maps, core_ids=list(range(NCORES)), **kwargs)

    out = np.empty((B, S, D), np.float32)
    for c in range(NCORES):
        b, i = c // 4, c % 4
        out[b, i * SL:(i + 1) * SL, :] = res.results[c]["y"]
    if trace_dir:
        kernel.last_exec_time_ns = res.exec_time_ns
        kernel.last_trace = res.instructions_and_trace
    return out


# revision 29
# speedup vs baseline: 1.2316x; 1.2316x over previous
"""Multi-head attention + residual + LayerNorm on 8 Trainium2 NeuronCores.

Reference computation (B=2, S=2048, D=1024, H=16, HD=64):
    q,k,v = split_heads(x@Wq+bq), ...       # [B,H,S,HD]
    attn  = softmax(q k^T / sqrt(HD))
    out   = (attn v) merged -> [B,S,D] @ Wp + bp
    y     = LayerNorm(x + out) * gamma + beta

Sharding: 8 cores = 2 batches x 4 query-slices of 512 rows.
Each core computes QKV projections for its 512-row slice; K^T (bf16) and
V (fp8e4) slices are AllGathered across the 4 cores of the same batch in
8 per-head-pair pieces fired as soon as each head pair's K/V is ready, so
attention pipelines behind the gathers. Each core runs attention for all
16 heads restricted to its 512 queries, then projection + residual + LN.

Softmax numerators are fp8e4: even-ish key chunks go through the scalar
engine's exact ACT exp, the rest through a one-pass Schraudolph bit-trick
on the vector engine (f32 -> uint8 code with round+saturate, bitcast as
fp8e4). attn@V runs as fp8 DoubleRow matmuls (two 128-key chunks per
pass). Residual + LayerNorm in f32.
"""

import os

import ml_dtypes
import numpy as np

import concourse.bacc as bacc
import concourse.tile as tile
from concourse import mybir
from concourse.bass_utils import run_bass_kernel_spmd

B, S, D, H, HD = 2, 2048, 1024, 16, 64
EPS = 1e-5
NCORES = 8
SL = S // 4          # 512 query rows per core
GROUPS = [[0, 1, 2, 3], [4, 5, 6, 7]]
BF = mybir.dt.bfloat16
F32 = mybir.dt.float32
FP8 = mybir.dt.float8e4
U8 = mybir.dt.uint8
I16 = mybir.dt.int16
Act = mybir.ActivationFunctionType
Alu = mybir.AluOpType
DR = mybir.MatmulPerfMode.DoubleRow

# exp(s*0.125 + BSH); uniform shift cancels in softmax, keeps fp8 in range
BSH = -1.2
LOG2E = 1.4426950408889634
# one-pass Schraudolph to fp8e4 codes: u8 = round(s*EC1 + EC2), bitcast e4m3
# (negatives saturate to 0 == fp8 +0.0, so no NaN poisoning)
EC1 = 8 * LOG2E * 0.125
EC2 = 56.0 + 8 * LOG2E * BSH + 0.34
# kc chunks handled by the scalar engine's exact ACT exp (rest: DVE trick)
SCALAR_KCS = {0, 2, 4, 6, 8, 10, 12, 14}

KB = 128 * SL            # 65536 bf16 elems: K^T block per head pair
VB = 128 * SL // 2       # V block as bf16-slot count (fp8 bytes = 128*SL)
KVB = KB + VB            # 98304


def build_program():
    nc = bacc.Bacc("TRN2", target_bir_lowering=False, debug=False,
                   num_devices=NCORES)

    # ---- I/O ----
    xT_d = nc.dram_tensor("xT", [D, SL], BF, kind="ExternalInput")
    xq_d = nc.dram_tensor("xq", [SL, D], F32, kind="ExternalInput")
    wq_d = nc.dram_tensor("wq", [D, D], BF, kind="ExternalInput")
    wk_d = nc.dram_tensor("wk", [D, D], BF, kind="ExternalInput")
    wv_d = nc.dram_tensor("wv", [D, D], BF, kind="ExternalInput")
    wp_d = nc.dram_tensor("wp", [D, D], FP8, kind="ExternalInput")
    bq_d = nc.dram_tensor("bq", [D], F32, kind="ExternalInput")
    bk_d = nc.dram_tensor("bk", [D], F32, kind="ExternalInput")
    bv_d = nc.dram_tensor("bv", [D], F32, kind="ExternalInput")
    bp_d = nc.dram_tensor("bp", [D], BF, kind="ExternalInput")
    gamma_d = nc.dram_tensor("gamma", [D], F32, kind="ExternalInput")
    beta_d = nc.dram_tensor("beta", [D], F32, kind="ExternalInput")
    y_d = nc.dram_tensor("y", [SL, D], F32, kind="ExternalOutput")

    import concourse.bass as bass

    def bcast_ap(dram_t, parts=128):
        # replicate a [D] dram vector across `parts` partitions
        return bass.AP(tensor=dram_t, offset=0, ap=[[0, parts], [1, D]])

    with tile.TileContext(nc) as tc:
        with (
            tc.tile_pool(name="persist", bufs=1) as persist,
            tc.tile_pool(name="dram", bufs=1, space="DRAM") as dram,
        ):
            # persistent tiles
            qt_sb = persist.tile([128, 8, SL], BF)        # Q^T
            xq_sb = persist.tile([128, 4, D], F32)        # natural x slice
            wp_sb = persist.tile([128, 8, 2, 512], FP8)
            outT_sb = persist.tile([128, 8, SL], FP8)     # normalized attn out^T
            bv_bc = persist.tile([128, D], F32)
            gamma_bc = persist.tile([128, D], F32)
            beta_bc = persist.tile([128, D], F32)
            bq_sb = persist.tile([128, 8], F32)
            bk_sb = persist.tile([128, 8], F32)
            bp_sb = persist.tile([1, D], BF)
            ones_sb = persist.tile([1, 128], BF)
            eps_sb = persist.tile([128, 1], F32)
            bsh_sb = persist.tile([128, 1], F32)
            # V tiles, fp8, interleaved kc pairs for DoubleRow:
            # [part=key-in-chunk, kc-pair, kc-parity, 144] with
            # cols 0:64 = V_A, 64 = ones, 65:129 = V_B, 129 = ones
            vh_tiles = [persist.tile([128, 8, 2, 144], FP8, name=f"vh{i}")
                        for i in range(2)]

            # DRAM scratch per head pair: K^T bf16 (p-major, KB elems) then
            # V fp8 (s-major, VB bf16 slots) in one bf16 tensor so a single
            # AllGather moves both.
            kvb_hp = [dram.tile([1, KVB], BF, name=f"kvb{i}") for i in range(8)]
            kvg_hp = [dram.tile([4, KVB], BF, name=f"kvg{i}") for i in range(8)]
            dumb_in = dram.tile([1, 128], BF, name="dumb_in")
            dumb_out = dram.tile([4, 128], BF, name="dumb_out")
            nc.gpsimd.collective_compute(
                "AllGather", Alu.bypass, replica_groups=GROUPS,
                ins=[dumb_in[:].opt()], outs=[dumb_out[:].opt()])

            # small/early loads first (biases feed phase-1 epilogues)
            nc.sync.dma_start(bk_sb[:], bk_d.ap().rearrange("(co p) -> p co", p=128))
            nc.sync.dma_start(bv_bc[:], bcast_ap(bv_d))
            nc.sync.dma_start(bq_sb[:], bq_d.ap().rearrange("(co p) -> p co", p=128))
            nc.vector.memset(ones_sb[:], 1.0)
            nc.vector.memset(eps_sb[:], EPS)
            nc.vector.memset(bsh_sb[:], BSH)
            for t in vh_tiles:
                nc.vector.memset(t[:, :, :, 64:65], 1.0)
                nc.vector.memset(t[:, :, :, 129:130], 1.0)
            def kvb_kview(hp):
                return kvb_hp[hp][0, 0:KB].rearrange("(p s) -> p s", p=128)

            def kvb_vview(hp):
                return kvb_hp[hp][0, KB:KVB].bitcast(FP8).rearrange(
                    "(s c) -> s c", c=128)

            # ---------------- phase 1: QKV projections for this slice ----------------
            with (
                tc.tile_pool(name="ph1w", bufs=1) as ph1w,
                tc.tile_pool(name="ph1", bufs=3) as ph1,
                tc.tile_pool(name="psum1", bufs=2, space="PSUM") as psum1,
            ):
                # x^T chunks first (every matmul needs them), then weights in
                # per-head-pair-group need-order so gathers can fire early.
                xt_c = []
                for ci in range(8):
                    xt = ph1w.tile([128, SL], BF, tag=f"xt{ci}")
                    nc.sync.dma_start(xt[:], xT_d[ci * 128:(ci + 1) * 128, :])
                    xt_c.append(xt)
                wkp, wvp = {}, {}
                for g in range(4):
                    for ci in range(8):
                        wk = ph1w.tile([128, 2, 128], BF, tag=f"wk{g}_{ci}")
                        nc.sync.dma_start(
                            wk[:], wk_d[ci * 128:(ci + 1) * 128, 256 * g:256 * (g + 1)]
                            .rearrange("p (co q) -> p co q", q=128))
                        wkp[(g, ci)] = wk
                    for ci in range(8):
                        wv = ph1w.tile([128, 256], BF, tag=f"wv{g}_{ci}")
                        nc.sync.dma_start(
                            wv[:], wv_d[ci * 128:(ci + 1) * 128, 256 * g:256 * (g + 1)])
                        wvp[(g, ci)] = wv
                wq_c = []
                for ci in range(8):
                    wq = ph1w.tile([128, 8, 128], BF, tag=f"wq{ci}")
                    nc.sync.dma_start(wq[:], wq_d[ci * 128:(ci + 1) * 128, :]
                                      .rearrange("p (co q) -> p co q", q=128))
                    wq_c.append(wq)
                # phase-3 loads last on the sync ring so they don't starve
                # the phase-1 critical loads above
                nc.sync.dma_start(xq_sb[:], xq_d.ap().rearrange("(qc p) d -> p qc d", p=128))
                nc.sync.dma_start(wp_sb[:], wp_d.ap().rearrange("(hp p) (dh q) -> p hp dh q", p=128, q=512))
                nc.sync.dma_start(gamma_bc[:], bcast_ap(gamma_d))
                nc.sync.dma_start(beta_bc[:], bcast_ap(beta_d))
                nc.sync.dma_start(bp_sb[:], bp_d.ap().rearrange("(o d) -> o d", o=1))

                def k_chunk(g, half):
                    co = 2 * g + half
                    ps = psum1.tile([128, SL], F32, tag="psk", name=f"psk{co}")
                    for ci in range(8):
                        nc.tensor.matmul(ps[:], wkp[(g, ci)][:, half, :], xt_c[ci][:],
                                         start=(ci == 0), stop=(ci == 7))
                    kt_t = ph1.tile([128, SL], BF, tag="kt", name=f"ktt{co}")
                    nc.vector.tensor_scalar_add(kt_t[:], ps[:], bk_sb[:, co:co + 1])
                    nc.gpsimd.dma_start(kvb_kview(co), kt_t[:])

                def v_pair(g):
                    ps = psum1.tile([128, 4, 256], F32, tag="psv", name=f"psv{g}")
                    for sc in range(4):
                        for ci in range(8):
                            nc.tensor.matmul(ps[:, sc, :],
                                             xt_c[ci][:, sc * 128:(sc + 1) * 128],
                                             wvp[(g, ci)][:],
                                             start=(ci == 0), stop=(ci == 7))
                    v_t = ph1.tile([128, 4, 256], FP8, tag="vt", name=f"vtt{g}")
                    with nc.allow_low_precision("V in fp8 for DoubleRow PV"):
                        for sc in range(4):
                            nc.vector.tensor_add(v_t[:, sc, :], ps[:, sc, :],
                                                 bv_bc[:, 256 * g:256 * (g + 1)])
                    for half in range(2):
                        hp = 2 * g + half
                        vv = kvb_vview(hp)
                        for sc in range(4):
                            nc.gpsimd.dma_start(
                                vv[sc * 128:(sc + 1) * 128, :],
                                v_t[:, sc, 128 * half:128 * (half + 1)])

                def fire_cc(hp):
                    nc.gpsimd.collective_compute(
                        "AllGather", Alu.bypass, replica_groups=GROUPS,
                        ins=[kvb_hp[hp][:].opt()], outs=[kvg_hp[hp][:].opt()])

                for g in range(4):
                    k_chunk(g, 0)
                    k_chunk(g, 1)
                    v_pair(g)
                    fire_cc(2 * g)
                    fire_cc(2 * g + 1)

                # Q^T (local only; overlaps the gathers)
                for co in range(8):
                    ps = psum1.tile([128, SL], F32, tag="psk", name=f"psq{co}")
                    for ci in range(8):
                        nc.tensor.matmul(ps[:], wq_c[ci][:, co, :], xt_c[ci][:],
                                         start=(ci == 0), stop=(ci == 7))
                    nc.vector.tensor_scalar_add(qt_sb[:, co, :], ps[:], bq_sb[:, co:co + 1])

            # ---------------- phase 2: attention, one head pair at a time ----------------
            with (
                tc.tile_pool(name="kv", bufs=2) as kv,
                tc.tile_pool(name="expp", bufs=2) as expp,
                tc.tile_pool(name="small", bufs=3) as small,
                tc.tile_pool(name="ps_sc", bufs=2, space="PSUM") as ps_sc,
                tc.tile_pool(name="ps_o", bufs=1, space="PSUM") as ps_o,
                tc.tile_pool(name="ps_rb", bufs=1, space="PSUM") as ps_rb,
            ):
                for hp in range(8):
                    kth_t = kv.tile([128, 16, 128], BF, tag="kth")
                    vh_t = vh_tiles[hp % 2]
                    for j in range(4):
                        ksrc = kvg_hp[hp][j, 0:KB].rearrange(
                            "(p k4 m) -> p k4 m", p=128, m=128)
                        nc.sync.dma_start(kth_t[:, 4 * j:4 * (j + 1), :], ksrc)
                        vsrc = kvg_hp[hp][j, KB:KVB].bitcast(FP8).rearrange(
                            "(jp o p c) -> p jp o c", jp=2, o=2, p=128)
                        nc.sync.dma_start(vh_t[:, 2 * j:2 * j + 2, :, 0:64],
                                          vsrc[:, :, :, 0:64])
                        nc.sync.dma_start(vh_t[:, 2 * j:2 * j + 2, :, 65:129],
                                          vsrc[:, :, :, 64:128])

                    exp_t = expp.tile([128, 16, 2, 512], FP8, tag="exp")
                    oAB = ps_o.tile([65, 2, SL], F32, tag="oAB")

                    def pv_pair(j):
                        for h in range(2):
                            nc.tensor.matmul(oAB[:, h, :],
                                             vh_t[:, j, :, 65 * h:65 * h + 65],
                                             exp_t[:, 2 * j:2 * j + 2, h, :],
                                             start=(j == 0), stop=(j == 7),
                                             perf_mode=DR)

                    for kc in range(16):
                        ps = ps_sc.tile([128, 2, 512], F32, tag="sc")
                        # head A on PE rows 0-63, head B on rows 64-127 (row-tiled)
                        nc.tensor.matmul(ps[:, 0, :], kth_t[0:64, kc, :],
                                         qt_sb[0:64, hp, :], start=True, stop=True)
                        nc.tensor.matmul(ps[:, 1, :], kth_t[64:128, kc, :],
                                         qt_sb[64:128, hp, :], start=True, stop=True)
                        if kc in SCALAR_KCS:
                            nc.scalar.activation(exp_t[:, kc, :, :], ps[:], Act.Exp,
                                                 scale=0.125, bias=bsh_sb[:])
                        else:
                            with nc.allow_low_precision("schraudolph fp8 exp"):
                                nc.vector.tensor_scalar(
                                    exp_t[:, kc, :, :].bitcast(U8), ps[:],
                                    EC1, EC2, Alu.mult, Alu.add)
                        # attn@V for pair j once both its kc's exp are 2 behind
                        if kc >= 3 and kc % 2 == 1:
                            pv_pair((kc - 3) // 2)
                    pv_pair(6)
                    pv_pair(7)

                    # softmax normalization: rows 0-63 = head dims, row 64 = sum(exp)
                    sAB = small.tile([1, 2, SL], F32, tag="sAB")
                    nc.scalar.copy(sAB[:], oAB[64:65, :, :])
                    rABf = small.tile([1, 2, SL], F32, tag="rABf")
                    nc.vector.reciprocal_approx_fast(out=rABf[:], in_=sAB[:])
                    rAB = small.tile([1, 2, SL], BF, tag="rAB")
                    with nc.allow_low_precision("softmax scale in bf16"):
                        nc.vector.tensor_copy(rAB[:], rABf[:])
                    rbA = ps_rb.tile([64, SL], F32, tag="rbA")
                    rbB = ps_rb.tile([64, SL], F32, tag="rbB")
                    nc.tensor.matmul(rbA[:], ones_sb[0:1, 0:64], rAB[0:1, 0, :], start=True, stop=True)
                    nc.tensor.matmul(rbB[:], ones_sb[0:1, 0:64], rAB[0:1, 1, :], start=True, stop=True)
                    # rb psum -> sbuf so the final mult reads one PSUM operand
                    rbS = small.tile([64, 2, SL], F32, tag="rbS")
                    nc.vector.tensor_copy(rbS[:, 0, :], rbA[:])
                    nc.vector.tensor_copy(rbS[:, 1, :], rbB[:])
                    with nc.allow_low_precision("attn out in fp8 for DR proj"):
                        nc.vector.scalar_tensor_tensor(
                            out=outT_sb[0:64, hp, :], in0=oAB[0:64, 0, :], scalar=1.0,
                            in1=rbS[:, 0, :], op0=Alu.bypass, op1=Alu.mult)
                        tmpB = small.tile([64, SL], FP8, tag="tmpB")
                        nc.vector.scalar_tensor_tensor(
                            out=tmpB[:], in0=oAB[0:64, 1, :], scalar=1.0,
                            in1=rbS[:, 1, :], op0=Alu.bypass, op1=Alu.mult)
                    nc.gpsimd.dma_start(outT_sb[64:128, hp, :], tmpB[:])

            # ---------------- phase 3: out-projection + residual + LayerNorm ----------------
            with (
                tc.tile_pool(name="ph3", bufs=3) as ph3,
                tc.tile_pool(name="ph3s", bufs=4) as ph3s,
                tc.tile_pool(name="psum3", bufs=4, space="PSUM") as psum3,
            ):
                for qc in range(4):
                    y_t = ph3.tile([128, D], F32, tag="y")
                    for dh in range(2):
                        ps = psum3.tile([128, 512], F32, tag="py")
                        for t in range(4):
                            nc.tensor.matmul(ps[:],
                                             outT_sb[:, 2 * t:2 * t + 2, qc * 128:(qc + 1) * 128],
                                             wp_sb[:, 2 * t:2 * t + 2, dh, :],
                                             start=(t == 0), stop=False, perf_mode=DR)
                        # + bp via a rank-1 matmul with a ones row
                        nc.tensor.matmul(ps[:], ones_sb[0:1, :],
                                         bp_sb[0:1, dh * 512:(dh + 1) * 512],
                                         start=False, stop=True)
                        nc.vector.tensor_add(y_t[:, dh * 512:(dh + 1) * 512], ps[:],
                                             xq_sb[:, qc, dh * 512:(dh + 1) * 512])
                    # LayerNorm over D=1024
                    stats = ph3s.tile([128, 2, 6], F32, tag="stats")
                    nc.vector.bn_stats(stats[:, 0, :], y_t[:, 0:512])
                    nc.vector.bn_stats(stats[:, 1, :], y_t[:, 512:1024])
                    mv = ph3s.tile([128, 2], F32, tag="mv")
                    nc.vector.bn_aggr(mv[:], stats[:])
                    rstd = ph3s.tile([128, 1], F32, tag="rstd")
                    nc.scalar.activation(rstd[:], mv[:, 1:2], Act.Sqrt, bias=eps_sb[:])
                    nc.vector.reciprocal(rstd[:], rstd[:])
                    # y = ((y - mu) * gamma) * rstd + beta
                    nc.vector.scalar_tensor_tensor(
                        out=y_t[:], in0=y_t[:], scalar=mv[:, 0:1], in1=gamma_bc[:],
                        op0=Alu.subtract, op1=Alu.mult)
                    nc.vector.scalar_tensor_tensor(
                        out=y_t[:], in0=y_t[:], scalar=rstd[:], in1=beta_bc[:],
                        op0=Alu.mult, op1=Alu.add)
                    nc.sync.dma_start(y_d[qc * 128:(qc + 1) * 128, :], y_t[:])

    nc.compile()
    return nc


_PROGRAM = None


def _get_program():
    global _PROGRAM
    if _PROGRAM is None:
        _PROGRAM = build_program()
    return _PROGRAM


def kernel(**inputs):
    x = np.asarray(inputs["x"], np.float32)
    bf = ml_dtypes.bfloat16
    shared = {
        "wq": np.asarray(inputs["Wq"], np.float32).astype(bf),
        "wk": np.asarray(inputs["Wk"], np.float32).astype(bf),
        "wv": np.asarray(inputs["Wv"], np.float32).astype(bf),
        "wp": np.asarray(inputs["Wp"], np.float32).astype(ml_dtypes.float8_e4m3fn),
        "bq": np.asarray(inputs["bq"], np.float32),
        "bk": np.asarray(inputs["bk"], np.float32),
        "bv": np.asarray(inputs["bv"], np.float32),
        "bp": np.asarray(inputs["bp"], np.float32).astype(bf),
        "gamma": np.asarray(inputs["gamma"], np.float32),
        "beta": np.asarray(inputs["beta"], np.float32),
    }
    in_maps = []
    for c in range(NCORES):
        b, i = c // 4, c % 4
        xs = np.ascontiguousarray(x[b, i * SL:(i + 1) * SL, :])
        m = dict(shared)
        m["xT"] = np.ascontiguousarray(xs.T).astype(bf)
        m["xq"] = xs
        in_maps.append(m)

    nc = _get_program()
    trace_dir = os.environ.get("BASS_KERNEL_TRACE_DIR")
    kwargs = {}
    if trace_dir:
        kwargs = {"trace": True, "tmpdir": trace_dir}
    res = run_bass_kernel_spmd(nc, in_maps, core_ids=list(range(NCORES)), **kwargs)

    out = np.empty((B, S, D), np.float32)
    for c in range(NCORES):
        b, i = c // 4, c % 4
        out[b, i * SL:(i + 1) * SL, :] = res.results[c]["y"]
    if trace_dir:
        kernel.last_exec_time_ns = res.exec_time_ns
        kernel.last_trace = res.instructions_and_trace
    return out
